# revision 1
# baseline (speedup 1.0000x reference)
"""Conv2d 3x3 (stride 1, pad 1) as implicit GEMM on 8 Trainium2 NeuronCores.

x: [32, 128, 56, 56] f32, W: [256, 128, 3, 3] f32 -> out: [32, 256, 56, 56] f32

Sharding: data-parallel over batch, 4 images per core (sharding_hint).

Per-core kernel (PE-bound, ~97us of bf16 matmul at 78.6 TF/s peak):
  - host pre-pads x to [4, 128, 58, 58], casts to bf16; pre-transposes W to
    [Cin=128, 9*Cout] bf16 (tap-major) so no on-device transposes are needed
  - Cin=128 is the contraction dim and lives on the SBUF partition axis; for
    each output tile (img, 8-row group, cout half) nine matmuls (one per
    3x3 tap, N=448 columns) accumulate into one PSUM bank, with the padded
    input addressed through strided [128, 8, 56] views (no im2col copies)
  - PSUM -> SBUF fp32 copy on the vector engine, streaming HWDGE store per
    tile; input DMAs ride both HWDGE rings, image 0 split into row chunks so
    compute starts ~1us in; a chain of dependency-free warmup matmuls holds
    the PE clock ramp (HAM) warm while the first loads land
  - built on bacc.Bacc so multi-wait instructions are legalized (split) for
    the 1-sync-wait-per-instruction encoding limit of this toolchain

Measured: TimelineSim (repo cost model) 103.2us single-shot; real-HW
steady-state body ~90us via repeated-body slope timing (NTFF profiling is
unavailable under this axon build). Numerics: bf16 inputs / fp32 PSUM
accumulate -> rel L2 error ~2.1e-3 vs the fp32 reference.
"""

import sys

for _p in ("/opt/trn_rl_repo",):
    if _p not in sys.path:
        sys.path.insert(0, _p)

import numpy as np
import ml_dtypes

import concourse.bass as bass
import concourse.bacc as bacc
import concourse.mybir as mybir
from concourse import tile
from concourse.bass_utils import run_bass_kernel_spmd

N_CORES = 8
B = 32
B_PER_CORE = B // N_CORES  # 4
CIN = 128
COUT = 256
H = W_DIM = 56
HP = WP = 58  # padded
KH = KW = 3
KPOS = KH * KW  # 9
ROWS = 8               # output rows per matmul
NG = H // ROWS         # 7 row groups
NFREE = ROWS * W_DIM   # 448 free dim per matmul (<= 512 psum bank)
COUT_TILES = COUT // 128  # 2

_NC_CACHE = None


def build_nc(reps: int = 1, xsplits=(0, 10, 18, 34, HP), wchunks: int = 2) -> bass.Bass:
    # Bacc (not raw Bass): its compile() runs move_matmul_waits_to_ldweights
    # and generate_event_semaphores, which split multi-wait instructions to
    # satisfy the 1-sync-wait-per-instruction hardware encoding limit.
    # reps > 1 repeats the compute+store body (same outputs) for slope-based
    # hardware timing; the shipped kernel uses reps=1.
    nc = bacc.Bacc()
    xp = nc.dram_tensor(
        "xp", [B_PER_CORE, CIN, HP * WP], mybir.dt.bfloat16, kind="ExternalInput"
    )
    wt = nc.dram_tensor(
        "wt", [CIN, KPOS * COUT], mybir.dt.bfloat16, kind="ExternalInput"
    )
    out = nc.dram_tensor(
        "out", [B_PER_CORE, COUT, H * W_DIM], mybir.dt.float32, kind="ExternalOutput"
    )

    with tile.TileContext(nc) as tc:
        with (
            tc.tile_pool(name="wpool", bufs=1) as wpool,
            tc.tile_pool(name="xpool", bufs=1) as xpool,
            tc.tile_pool(name="opool", bufs=6) as opool,
            tc.tile_pool(name="pspool", bufs=7, space="PSUM") as pspool,
            tc.tile_pool(name="warmpool", bufs=1, space="PSUM") as warmpool,
        ):
            # Warm the PE clock (HAM / p-state ramp) while the input DMAs are
            # in flight: a chain of dependency-free matmuls on a memset
            # scratch tile keeps the PE busy from t=0, so the real matmuls
            # start at full clock. These never block the real stream (they
            # are ahead of it in PE program order and wait on nothing).
            scratch = opool.tile([128, 64], mybir.dt.bfloat16, name="warm_src", tag="wsrc")
            nc.vector.memset(scratch, 0.0)
            warm_ps = warmpool.tile([64, 64], mybir.dt.float32, name="warm_ps", tag="wps")
            for _ in range(64):
                nc.tensor.matmul(warm_ps, scratch[:, :64], scratch, start=True, stop=True)
            # Loads ride both HWDGE rings in parallel: weights (2 chunks) on
            # the scalar ring, x images (3 row chunks each) on the sync ring.
            # Chunking lets the first matmuls start as soon as weight chunk 0
            # and rows 0..17 of image 0 have landed; row chunk boundaries are
            # aligned so row group g only reads padded rows [8g, 8g+9].
            w_sb = wpool.tile([CIN, KPOS * COUT], mybir.dt.bfloat16, name="w_sb")
            WSPLITS = tuple(
                (KPOS * COUT) * i // wchunks for i in range(wchunks)
            ) + (KPOS * COUT,)
            for lo, hi in zip(WSPLITS[:-1], WSPLITS[1:]):
                nc.scalar.dma_start(w_sb[:, lo:hi], wt[:, lo:hi])

            x_views = []
            for b in range(B_PER_CORE):
                xb = xpool.tile(
                    [CIN, HP * WP], mybir.dt.bfloat16, name=f"x_sb{b}", tag=f"x{b}"
                )
                # Only image 0 races the PE; later images load as one DMA.
                splits = tuple(xsplits) if b == 0 else (0, HP)
                for lo, hi in zip(splits[:-1], splits[1:]):
                    nc.sync.dma_start(
                        xb[:, lo * WP : hi * WP], xp[b, :, lo * WP : hi * WP]
                    )
                x_views.append(xb.rearrange("p (h w) -> p h w", w=WP))

            for _rep in range(reps):
              for b in range(B_PER_CORE):
                for g in range(NG):
                    for c in range(COUT_TILES):
                        r0 = g * ROWS
                        ps = pspool.tile(
                            [128, NFREE], mybir.dt.float32, name="ps", tag="ps"
                        )
                        for k in range(KPOS):
                            kh, kw = divmod(k, KW)
                            rhs = x_views[b][:, r0 + kh : r0 + kh + ROWS, kw : kw + W_DIM]
                            lhsT = w_sb[:, k * COUT + c * 128 : k * COUT + (c + 1) * 128]
                            nc.tensor.matmul(
                                ps, lhsT, rhs, start=(k == 0), stop=(k == KPOS - 1)
                            )
                        ob = opool.tile(
                            [128, NFREE], mybir.dt.float32, name="ob", tag="ob"
                        )
                        nc.vector.tensor_copy(ob, ps)
                        nc.sync.dma_start(
                            out[
                                b,
                                c * 128 : (c + 1) * 128,
                                r0 * W_DIM : (r0 + ROWS) * W_DIM,
                            ],
                            ob,
                        )
    nc.compile()
    return nc


def _get_nc() -> bass.Bass:
    global _NC_CACHE
    if _NC_CACHE is None:
        _NC_CACHE = build_nc()
    return _NC_CACHE


def _prep_inputs(x: np.ndarray, W: np.ndarray):
    x = np.asarray(x, dtype=np.float32)
    W = np.asarray(W, dtype=np.float32)
    bf16 = ml_dtypes.bfloat16

    xp = np.zeros((B, CIN, HP, WP), dtype=bf16)
    xp[:, :, 1 : 1 + H, 1 : 1 + W_DIM] = x.astype(bf16)
    xp = xp.reshape(B, CIN, HP * WP)

    # Wt[ci, k*COUT + co] = W[co, ci, kh, kw], k = kh*3 + kw
    Wt = (
        W.transpose(2, 3, 1, 0)          # [kh, kw, ci, co]
        .reshape(KPOS, CIN, COUT)        # [k, ci, co]
        .transpose(1, 0, 2)              # [ci, k, co]
        .reshape(CIN, KPOS * COUT)
        .astype(bf16)
    )

    in_maps = []
    for c in range(N_CORES):
        in_maps.append(
            {
                "xp": np.ascontiguousarray(xp[c * B_PER_CORE : (c + 1) * B_PER_CORE]),
                "wt": Wt,
            }
        )
    return in_maps


def kernel_run(x: np.ndarray, W: np.ndarray, **spmd_kwargs):
    """Run the conv and return (output, BassKernelResults)."""
    in_maps = _prep_inputs(x, W)
    res = run_bass_kernel_spmd(
        _get_nc(), in_maps, core_ids=list(range(N_CORES)), **spmd_kwargs
    )
    out = np.concatenate(
        [
            np.asarray(res.results[c]["out"], dtype=np.float32).reshape(
                B_PER_CORE, COUT, H, W_DIM
            )
            for c in range(N_CORES)
        ],
        axis=0,
    )
    return out, res


def kernel(x: np.ndarray, W: np.ndarray) -> np.ndarray:
    out, _ = kernel_run(x, W)
    return out



# revision 15
# speedup vs baseline: 1.0084x; 1.0084x over previous
"""Conv2d 3x3 (stride 1, pad 1) as implicit GEMM on 8 Trainium2 NeuronCores.

x: [32, 128, 56, 56] f32, W: [256, 128, 3, 3] f32 -> out: [32, 256, 56, 56] f32

Sharding: data-parallel over batch, 4 images per core (sharding_hint).

Per-core kernel (PE-bound, ~94us of bf16 matmul at 78.6 TF/s peak):
  - host pre-pads x to [4, 128, 58, 58], casts to bf16; pre-transposes W to
    [Cin=128, 2*9*128] bf16 (cout-half-major, tap-minor) so no on-device
    transposes are needed and each cout half is one contiguous DMA chunk
  - Cin=128 is the contraction dim and lives on the SBUF partition axis; for
    each output tile (img, cout half, 8-row group) nine matmuls (one per
    3x3 tap, N=448 columns) accumulate into one PSUM bank, with the padded
    input addressed through strided [128, 8, 56] views (no im2col copies)
  - loop order (img, cout-half, row-group) so the second weight half is not
    needed until ~16us in; first weight half + first x rows ride the sync
    HWDGE ring back-to-back so the first real matmul issues ~4us in, right
    as the PE p-state ramp (3us continuous busy) completes behind a chain of
    dependency-free warmup matmuls
  - PSUM -> SBUF fp32 copy on the vector engine, streaming HWDGE store per
    tile; the last tile is split column-wise so the final copy+store drain
    after the last matmul is short
  - built on bacc.Bacc so multi-wait instructions are legalized (split) for
    the 1-sync-wait-per-instruction encoding limit of this toolchain

Numerics: bf16 inputs / fp32 PSUM accumulate -> rel L2 error ~2.1e-3 vs the
fp32 reference.
"""

import sys

for _p in ("/opt/trn_rl_repo",):
    if _p not in sys.path:
        sys.path.insert(0, _p)

import numpy as np
import ml_dtypes

import concourse.bass as bass
import concourse.bacc as bacc
import concourse.mybir as mybir
from concourse import tile
from concourse.bass_utils import run_bass_kernel_spmd

N_CORES = 8
B = 32
B_PER_CORE = B // N_CORES  # 4
CIN = 128
COUT = 256
H = W_DIM = 56
HP = WP = 58  # padded
KH = KW = 3
KPOS = KH * KW  # 9
ROWS = 8               # output rows per matmul
NG = H // ROWS         # 7 row groups
NFREE = ROWS * W_DIM   # 448 free dim per matmul (<= 512 psum bank)
COUT_TILES = COUT // 128  # 2
WHALF = KPOS * 128     # 1152 cols per cout half in the transposed weight

_NC_CACHE = None


PRE_WTAPS = 3            # weight taps (cout half 0) leading the prelude
PRE_XCOL = 7 * 58        # x rows start here (PRE_WTAPS*128=384 padded to 406)
PRE_XROWS = 10           # x image-0 rows in the prelude (g0 needs rows 0..9)
PRE_COLS = PRE_XCOL + PRE_XROWS * 58  # 986 = 17*58, rearrangeable as (h w)


def build_nc(
    reps: int = 1,
    xsplits=(0, 18, 34, 58),
    warm=(27, 128),
    tail_split=168,
) -> bass.Bass:
    # Bacc (not raw Bass): its compile() runs move_matmul_waits_to_ldweights
    # and generate_event_semaphores, which split multi-wait instructions to
    # satisfy the 1-sync-wait-per-instruction hardware encoding limit.
    nc = bacc.Bacc()
    xp = nc.dram_tensor(
        "xp", [B_PER_CORE, CIN, HP * WP], mybir.dt.bfloat16, kind="ExternalInput"
    )
    wt = nc.dram_tensor(
        "wt", [CIN, COUT_TILES * WHALF], mybir.dt.bfloat16, kind="ExternalInput"
    )
    pre = nc.dram_tensor(
        "pre", [CIN, PRE_COLS], mybir.dt.bfloat16, kind="ExternalInput"
    )
    out = nc.dram_tensor(
        "out", [B_PER_CORE, COUT, H * W_DIM], mybir.dt.float32, kind="ExternalOutput"
    )

    nwarm, warm_cols = warm

    with tile.TileContext(nc) as tc:
        with (
            tc.tile_pool(name="wpool", bufs=1) as wpool,
            tc.tile_pool(name="xpool", bufs=1) as xpool,
            tc.tile_pool(name="opool", bufs=8) as opool,
            tc.tile_pool(name="pspool", bufs=7, space="PSUM") as pspool,
            tc.tile_pool(name="warmpool", bufs=1, space="PSUM") as warmpool,
        ):
            # Warm the PE clock (p-state ramp: full speed only after 3us of
            # continuous execution) while the input DMAs are in flight. The
            # chain is dependency-free so it never blocks the real stream.
            scratch = opool.tile([128, warm_cols], mybir.dt.bfloat16, name="warm_src", tag="wsrc")
            nc.vector.memset(scratch, 0.0)
            warm_ps = warmpool.tile([64, warm_cols], mybir.dt.float32, name="warm_ps", tag="wps")
            for _ in range(nwarm):
                nc.tensor.matmul(warm_ps, scratch[:, :64], scratch, start=True, stop=True)

            # All loads ride ONE ring (sync/SP) in strict need-order: the DMA
            # engines are a serial resource with no priority, so a big
            # transfer issued on another ring head-of-line-blocks urgently
            # needed chunks (and a stalled PE resets the p-state ramp).
            # The prelude packs x0 rows 0..9 + the first PRE_WTAPS weight taps
            # into one DMA so the first matmul's data arrives in one chain.
            pre_sb = wpool.tile([CIN, PRE_COLS], mybir.dt.bfloat16, name="pre_sb")
            pre_view = pre_sb.rearrange("p (h w) -> p h w", w=WP)
            w_sb = wpool.tile([CIN, COUT_TILES * WHALF], mybir.dt.bfloat16, name="w_sb")
            x_views = []
            x_tiles = []
            for b in range(B_PER_CORE):
                xb = xpool.tile(
                    [CIN, HP * WP], mybir.dt.bfloat16, name=f"x_sb{b}", tag=f"x{b}"
                )
                x_tiles.append(xb)
                x_views.append(xb.rearrange("p (h w) -> p h w", w=WP))

            nc.sync.dma_start(pre_sb, pre[:, 0:PRE_COLS])
            nc.sync.dma_start(
                w_sb[:, PRE_WTAPS * 128 : WHALF], wt[:, PRE_WTAPS * 128 : WHALF]
            )
            for lo, hi in zip(xsplits[:-1], xsplits[1:]):
                nc.sync.dma_start(
                    x_tiles[0][:, lo * WP : hi * WP], xp[0, :, lo * WP : hi * WP]
                )
            nc.sync.dma_start(w_sb[:, WHALF:], wt[:, WHALF:])
            nc.sync.dma_start(x_tiles[1], xp[1])
            nc.sync.dma_start(x_tiles[2], xp[2])
            nc.sync.dma_start(x_tiles[3], xp[3])

            def do_tile(b, c, row0, nrows, ps_name, use_pre=False):
                ncols = nrows * W_DIM
                ps = pspool.tile([128, ncols], mybir.dt.float32, name=ps_name, tag="ps")
                for k in range(KPOS):
                    kh, kw = divmod(k, KW)
                    if use_pre:
                        rhs = pre_view[
                            :, 7 + row0 + kh : 7 + row0 + kh + nrows, kw : kw + W_DIM
                        ]
                    else:
                        rhs = x_views[b][
                            :, row0 + kh : row0 + kh + nrows, kw : kw + W_DIM
                        ]
                    if c == 0 and k < PRE_WTAPS:
                        lhsT = pre_sb[:, k * 128 : (k + 1) * 128]
                    else:
                        lhsT = w_sb[:, c * WHALF + k * 128 : c * WHALF + (k + 1) * 128]
                    nc.tensor.matmul(
                        ps, lhsT, rhs, start=(k == 0), stop=(k == KPOS - 1)
                    )
                ob = opool.tile([128, ncols], mybir.dt.float32, name="ob", tag="ob")
                nc.vector.tensor_copy(ob, ps)
                nc.sync.dma_start(
                    out[
                        b,
                        c * 128 : (c + 1) * 128,
                        row0 * W_DIM : (row0 + nrows) * W_DIM,
                    ],
                    ob,
                )

            tail_rows = tail_split // W_DIM
            for _rep in range(reps):
                for b in range(B_PER_CORE):
                    for c in range(COUT_TILES):
                        last_bc = b == B_PER_CORE - 1 and c == COUT_TILES - 1
                        for g in range(NG):
                            if last_bc and tail_rows and g == NG - 1:
                                # Split the very last tile so the final store
                                # drain is short; the A part is sized so its
                                # copy/store clears each shared device (DVE,
                                # SP SEQ, HWDGE, DMA engines) just before the
                                # B part's chain arrives.
                                do_tile(b, c, g * ROWS, ROWS - tail_rows, "psA")
                                do_tile(
                                    b, c, g * ROWS + ROWS - tail_rows, tail_rows,
                                    "psB",
                                )
                            else:
                                do_tile(
                                    b, c, g * ROWS, ROWS, "ps",
                                    use_pre=(b == 0 and c == 0 and g == 0),
                                )
    nc.compile()
    return nc


def _get_nc() -> bass.Bass:
    global _NC_CACHE
    if _NC_CACHE is None:
        _NC_CACHE = build_nc()
    return _NC_CACHE


def _prep_inputs(x: np.ndarray, W: np.ndarray):
    x = np.asarray(x, dtype=np.float32)
    W = np.asarray(W, dtype=np.float32)
    bf16 = ml_dtypes.bfloat16

    xp = np.zeros((B, CIN, HP, WP), dtype=bf16)
    xp[:, :, 1 : 1 + H, 1 : 1 + W_DIM] = x.astype(bf16)
    xp = xp.reshape(B, CIN, HP * WP)

    # Wt[ci, c*1152 + k*128 + co'] = W[c*128 + co', ci, kh, kw], k = kh*3 + kw
    Wt = np.empty((CIN, COUT_TILES * WHALF), dtype=bf16)
    Wf = W.reshape(COUT, CIN, KPOS)               # [co, ci, k]
    for c in range(COUT_TILES):
        blk = Wf[c * 128 : (c + 1) * 128]         # [128co, ci, k]
        # -> [ci, k, co]
        Wt[:, c * WHALF : (c + 1) * WHALF] = (
            blk.transpose(1, 2, 0).reshape(CIN, WHALF).astype(bf16)
        )

    in_maps = []
    for c in range(N_CORES):
        xc = xp[c * B_PER_CORE : (c + 1) * B_PER_CORE]
        # Prelude: first PRE_WTAPS weight taps (cout half 0), then x image 0
        # rows 0..PRE_XROWS-1, padded so the tile rearranges as (h w), w=58.
        pre = np.zeros((CIN, PRE_COLS), dtype=bf16)
        pre[:, : PRE_WTAPS * 128] = Wt[:, : PRE_WTAPS * 128]
        pre[:, PRE_XCOL:] = xc[0, :, : PRE_XROWS * WP]
        in_maps.append(
            {
                "xp": np.ascontiguousarray(xc),
                "wt": Wt,
                "pre": pre,
            }
        )
    return in_maps


def kernel_run(x: np.ndarray, W: np.ndarray, **spmd_kwargs):
    """Run the conv and return (output, BassKernelResults)."""
    in_maps = _prep_inputs(x, W)
    res = run_bass_kernel_spmd(
        _get_nc(), in_maps, core_ids=list(range(N_CORES)), **spmd_kwargs
    )
    out = np.concatenate(
        [
            np.asarray(res.results[c]["out"], dtype=np.float32).reshape(
                B_PER_CORE, COUT, H, W_DIM
            )
            for c in range(N_CORES)
        ],
        axis=0,
    )
    return out, res


def kernel(x: np.ndarray, W: np.ndarray) -> np.ndarray:
    out, _ = kernel_run(x, W)
    return out
